# revision 42
# baseline (speedup 1.0000x reference)
"""GAT layer kernel for Trainium2, 8 NeuronCores.

Reference computation:
    X = node_features @ W            [N, DOUT]
    f0 = X @ v0 ; f1 = X @ v1       [N, 1]
    vals = sigmoid(f0 + f1.T) - 0.5
    alphas = softmax(where(graph != 0, vals, -inf), axis=1), masked to 0
    out = elu(alphas @ X)

Key identities / design:
  * sigmoid(z) - 0.5 = 0.5*tanh(z/2)  -> tanh and exp live in the same ACT
    table set (one table load; no per-tile sigmoid<->exp table switches).
  * softmax ratio: out_row = (sum_j m_ij e_ij X_j) / (sum_j m_ij e_ij) with
    e = exp(0.5*tanh(z/2)); the row-sum comes free as a ones-column in the
    matmul rhs, so the [N,N] attention matrix is never normalized at full
    width.
  * Row-sharding: each core owns N/8 output rows; softmax is row-wise so
    there is no cross-core reduction. Collectives measured ~90us of
    barrier+skew on this fabric, so instead of an AllGather each core
    recomputes X~ = nf @ [W | W@v0 | W@v1] for ALL rows from a replicated
    nf^T (DMA-cast f32->bf16 on load), pipelined in 128-row blocks so the
    attention pipeline starts as soon as block 0 lands.
  * The graph slice is shipped pre-transposed (host-side layout choice,
    same bytes, same device traffic) so the PE contraction dim (columns j)
    sits in partitions with plain contiguous loads.
"""

import numpy as np

import concourse.bass as bass
import concourse.mybir as mybir
import concourse.tile as tile
from concourse.bass_utils import run_bass_kernel_spmd

# ----------------------------------------------------------------------------
# Workaround for "Too many sync wait commands": this walrus build accepts only
# ONE sync-wait per instruction. Post-pass: hoist surplus waits onto
# single-wait NOPs on the same engine, inserted immediately before the
# instruction (identical blocking semantics, per-engine order preserved).
# ----------------------------------------------------------------------------


def _split_multi_waits(nc):
    import bass_rust

    eng = {
        mybir.EngineType.PE: nc.tensor,
        mybir.EngineType.DVE: nc.vector,
        mybir.EngineType.Activation: nc.scalar,
        mybir.EngineType.Pool: nc.gpsimd,
        mybir.EngineType.SP: nc.sync,
    }
    for f in nc.m.functions:
        for blk in f.blocks:
            fixups = []  # (index, inst, waits)
            for idx, inst in enumerate(blk.instructions):
                si = inst.sync_info
                waits = list(si.on_wait) if si is not None and si.on_wait else []
                if len(waits) > 1 and inst.engine in eng:
                    fixups.append((idx, inst, waits))
            if not fixups:
                continue
            nops_by_idx = {}
            created = set()
            for idx, inst, waits in fixups:
                inst.sync_info.on_wait = [waits[-1]]
                nops = []
                for w in waits[:-1]:
                    nop = eng[inst.engine].nop(nofuse=True, hint="wait_split").ins
                    nop.sync_info = bass_rust.SyncInfo(on_wait=[w], on_update=[])
                    nops.append(nop)
                    created.add(id(nop))
                nops_by_idx[idx] = nops
            # Drop the freshly-created nops from wherever nop() appended
            # them, then splice them in before their instruction.
            for b2 in f.blocks:
                b2.instructions[:] = [
                    i for i in b2.instructions if id(i) not in created
                ]
            new = []
            for idx, inst in enumerate(blk.instructions):
                new.extend(nops_by_idx.get(idx, ()))
                new.append(inst)
            blk.instructions[:] = new


# ----------------------------------------------------------------------------

F32 = mybir.dt.float32
BF16 = mybir.dt.bfloat16
FP8 = mybir.dt.float8e4
I32 = mybir.dt.int32
AF = mybir.ActivationFunctionType
ALU = mybir.AluOpType

N, D_IN, D_OUT = 8192, 512, 256
M_CORES = 8
P = 128


def build_gat(n=N, d_in=D_IN, d_out=D_OUT, m_cores=M_CORES, grp=4):
    """Per-core SPMD program. Inputs per core:
      graph_T  [n, R] int32   -- graph[rows].T (host-transposed slice)
      nfT_mine [d_in, R] f32  -- node_features[rows].T
      nfT_full [d_in, n] f32  -- node_features.T (replicated)
      wext     [d_in, d_out+2] f32 -- [W | W@v0 | W@v1] (replicated)
    Output: out [R, d_out] f32 (this core's rows)."""
    R = n // m_cores            # rows per core
    NJ = n // P                 # 128-wide j chunks over the full N
    IB = R // P                 # 128-row output blocks per core
    DK = d_in // P              # 128-deep contraction chunks
    DEXT = d_out + 2            # X | f0 | f1
    DW = d_out + 1              # main matmul rhs width: X | ones
    DPAD = ((DEXT + 15) // 16) * 16   # fp8 X~ row pitch (DoubleRow: step%16==0)
    n_grp = NJ // grp

    nc = bass.Bass(num_devices=m_cores)
    g_t = nc.declare_dram_parameter("graph_T", [n, R], I32, isOutput=False)
    nfT_mine = nc.declare_dram_parameter("nfT_mine", [d_in, R], F32, isOutput=False)
    nfT_full = nc.declare_dram_parameter("nfT_full", [d_in, n], F32, isOutput=False)
    wext = nc.declare_dram_parameter("wext", [d_in, DEXT], F32, isOutput=False)
    outp = nc.declare_dram_parameter("out", [R, d_out], F32, isOutput=True)

    with tile.TileContext(nc) as tc:
        with tc.tile_pool(name="persist", bufs=1) as persist, \
             tc.tile_pool(name="dram", bufs=1, space="DRAM") as dram, \
             tc.tile_pool(name="psum", bufs=1, space="PSUM") as psb, \
             tc.tile_pool(name="nfc", bufs=4) as nf_pool, \
             tc.tile_pool(name="mask", bufs=3) as mask_pool, \
             tc.tile_pool(name="tg", bufs=3) as t_pool, \
             tc.tile_pool(name="eg", bufs=3) as e_pool, \
             tc.tile_pool(name="pg", bufs=3) as p_pool, \
             tc.tile_pool(name="epi", bufs=2) as epi:

            # The 8 PSUM banks triple-duty: f0 mini-matmuls, X~ block
            # accumulation, then the 64-chunk attention accumulation.
            psum = [
                psb.tile([P, DEXT], F32, tag=f"ps{ib}", name=f"psum{ib}")
                for ib in range(IB)
            ]

            xsb = persist.tile([P, NJ, DEXT], BF16)      # X~ all rows (bf16)
            f0rep = persist.tile([P, R], F32)            # f0 row, replicated
            f1half = persist.tile([P, NJ], F32)          # 0.5*f1 per partition
            wextb = persist.tile([P, DK, DEXT], BF16)

            nc.gpsimd.dma_start(
                out=wextb,
                in_=bass.AP(wext, 0, [[DEXT, P], [P * DEXT, DK], [1, DEXT]]),
            )

            # ---- f0 for own rows, computed in ROW form ------------------
            # out[1, i] = sum_d wv0[d] nf[i, d]: stationary = wv0 (M=1), so
            # f0 lands as a row directly — no transpose, no column copies.
            wv0f = persist.tile([P, DK, 1], F32)
            nc.sync.dma_start(
                out=wv0f,
                in_=bass.AP(wext, d_out, [[DEXT, P], [P * DEXT, DK], [1, 1]]),
            )
            nfm = persist.tile([P, DK, R], F32)
            nc.sync.dma_start(
                out=nfm,
                in_=bass.AP(nfT_mine, 0, [[R, P], [P * R, DK], [1, R]]),
            )
            f0flat = persist.tile([1, R], F32)
            for ib in range(IB):
                ps = psum[ib]
                for kc in range(DK):
                    nc.tensor.matmul(
                        out=ps[0:1, 0:P],
                        lhsT=wv0f[:, kc, :],
                        rhs=nfm[:, kc, ib * P:(ib + 1) * P],
                        start=(kc == 0),
                        stop=(kc == DK - 1),
                    )
                nc.vector.tensor_copy(
                    out=f0flat[:, ib * P:(ib + 1) * P], in_=ps[0:1, 0:P]
                )
            f0dram = dram.tile([R], F32)
            nc.scalar.dma_start(
                out=f0dram.rearrange("(o r) -> o r", o=1), in_=f0flat
            )
            nc.scalar.dma_start(
                out=f0rep, in_=bass.AP(f0dram.tensor, 0, [[0, P], [1, R]])
            )

            # ---- X~ blocks for ALL rows (streamed, replicated compute) ---
            NFG = 4  # X~ blocks per nf load (bigger DMA runs)
            for gb in range(NJ // NFG):
                nfc = nf_pool.tile([P, DK, NFG * P], BF16)
                nc.gpsimd.dma_start(
                    out=nfc,
                    in_=bass.AP(
                        nfT_full, gb * NFG * P,
                        [[n, P], [P * n, DK], [1, NFG * P]],
                    ),
                )
                for bb in range(NFG):
                    ib = gb * NFG + bb
                    ps = psum[ib % IB]
                    for kc in range(DK):
                        mm = nc.tensor.matmul(
                            out=ps,
                            lhsT=nfc[:, kc, bb * P:(bb + 1) * P],
                            rhs=wextb[:, kc, :],
                            start=(kc == 0),
                            stop=(kc == DK - 1),
                        )

                    nc.vector.tensor_copy(
                        out=xsb[:, ib, 0:DEXT], in_=ps
                    )
                # batched per nf-group: 0.5*f1 slice + ones column
                nc.vector.tensor_scalar_mul(
                    f1half[:, gb * NFG:(gb + 1) * NFG],
                    xsb[:, gb * NFG:(gb + 1) * NFG, d_out + 1],
                    0.5,
                )
                nc.vector.memset(
                    xsb[:, gb * NFG:(gb + 1) * NFG, d_out], 1.0
                )

            # ---- masked-softmax attention matmul -------------------------
            for g in range(n_grp):
                # mask tile cast int32 -> bf16 in the DMA datapath (SWDGE)
                mbf = mask_pool.tile([P, grp, R], BF16)
                nc.gpsimd.dma_start(
                    out=mbf,
                    in_=bass.AP(g_t, g * grp * P * R, [[R, P], [P * R, grp], [1, R]]),
                )
                t_g = t_pool.tile([P, grp, R], BF16)
                for jj in range(grp):
                    jc = g * grp + jj
                    nc.scalar.activation(
                        out=t_g[:, jj, :],
                        in_=f0rep,
                        func=AF.Tanh,
                        bias=f1half[:, jc:jc + 1],
                        scale=0.5,
                    )
                e_g = e_pool.tile([P, grp, R], BF16)
                nc.scalar.activation(
                    out=e_g.rearrange("p g r -> p (g r)"),
                    in_=t_g.rearrange("p g r -> p (g r)"),
                    func=AF.Exp, scale=0.5,
                )
                p_g = p_pool.tile([P, grp, R], BF16)
                nc.vector.tensor_tensor(
                    out=p_g.rearrange("p g r -> p (g r)"),
                    in0=mbf.rearrange("p g r -> p (g r)"),
                    in1=e_g.rearrange("p g r -> p (g r)"),
                    op=ALU.mult,
                )
                for jj in range(grp):
                    jc = g * grp + jj
                    for ib in range(IB):
                        nc.tensor.matmul(
                            out=psum[ib][:, 0:DW],
                            lhsT=p_g[:, jj, ib * P:(ib + 1) * P],
                            rhs=xsb[:, jc, 0:DW],
                            start=(jc == 0),
                            stop=(jc == NJ - 1),
                        )

            # ---- epilogue: normalize + elu + store -----------------------
            for ib in range(IB):
                o = epi.tile([P, DW], F32, tag="o")
                nc.vector.tensor_copy(out=o, in_=psum[ib][:, 0:DW])
                sm = epi.tile([P, 1], F32, tag="sm")
                nc.vector.tensor_scalar_max(sm, o[:, d_out:DW], 1e-30)
                r = epi.tile([P, 1], F32, tag="r")
                nc.vector.reciprocal(out=r, in_=sm)
                u = epi.tile([P, d_out], F32, tag="u")
                nc.vector.tensor_scalar(
                    out=u, in0=o[:, 0:d_out], scalar1=r, scalar2=None,
                    op0=ALU.mult,
                )
                rp = epi.tile([P, d_out], F32, tag="rp")
                nc.vector.tensor_scalar_max(rp, u, 0.0)
                xm = epi.tile([P, d_out], F32, tag="xm")
                nc.vector.tensor_scalar_min(xm, u, 0.0)
                en = epi.tile([P, d_out], F32, tag="en")
                nc.scalar.activation(out=en, in_=xm, func=AF.Exp)
                res = epi.tile([P, d_out], F32, tag="res")
                nc.vector.tensor_tensor(out=res, in0=en, in1=rp, op=ALU.add)
                nc.vector.tensor_scalar_add(res, res, -1.0)
                nc.sync.dma_start(out=outp[ib * P:(ib + 1) * P, :], in_=res)

    _split_multi_waits(nc)
    return nc


_cached = {}

# Dev/test knobs (the grading harness just calls kernel(**inputs)):
_TRACE = False
_TMPDIR = None
_LAST_EXEC_NS = None
_LAST_RESULTS = None


def _get_program(n, d_in, d_out, m_cores):
    key = (n, d_in, d_out, m_cores)
    if key not in _cached:
        _cached[key] = build_gat(n, d_in, d_out, m_cores)
    return _cached[key]


def kernel(node_features, graph, W, v0, v1):
    node_features = np.asarray(node_features, dtype=np.float32)
    graph = np.ascontiguousarray(np.asarray(graph, dtype=np.int32))
    W = np.asarray(W, dtype=np.float32)
    v0 = np.asarray(v0, dtype=np.float32)
    v1 = np.asarray(v1, dtype=np.float32)

    n, d_in = node_features.shape
    d_out = W.shape[1]
    m = M_CORES
    R = n // m

    nc = _get_program(n, d_in, d_out, m)

    wext = np.concatenate([W, W @ v0, W @ v1], axis=1).astype(np.float32)
    nfT_full = np.ascontiguousarray(node_features.T)
    in_maps = []
    for c in range(m):
        rows = slice(c * R, (c + 1) * R)
        in_maps.append({
            "graph_T": np.ascontiguousarray(graph[rows].T),
            "nfT_mine": np.ascontiguousarray(node_features[rows].T),
            "nfT_full": nfT_full,
            "wext": wext,
        })
    global _LAST_EXEC_NS, _LAST_RESULTS
    res = run_bass_kernel_spmd(
        nc, in_maps, list(range(m)), trace=_TRACE, tmpdir=_TMPDIR
    )
    _LAST_EXEC_NS = res.exec_time_ns
    _LAST_RESULTS = res
    return np.concatenate([res.results[c]["out"] for c in range(m)], axis=0)


# revision 44
# speedup vs baseline: 1.0181x; 1.0181x over previous
"""GAT layer kernel for Trainium2, 8 NeuronCores.

Reference computation:
    X = node_features @ W            [N, DOUT]
    f0 = X @ v0 ; f1 = X @ v1       [N, 1]
    vals = sigmoid(f0 + f1.T) - 0.5
    alphas = softmax(where(graph != 0, vals, -inf), axis=1), masked to 0
    out = elu(alphas @ X)

Key identities / design:
  * sigmoid(z) - 0.5 = 0.5*tanh(z/2)  -> tanh and exp live in the same ACT
    table set (one table load; no per-tile sigmoid<->exp table switches).
  * softmax ratio: out_row = (sum_j m_ij e_ij X_j) / (sum_j m_ij e_ij) with
    e = exp(0.5*tanh(z/2)); the row-sum comes free as a ones-column in the
    matmul rhs, so the [N,N] attention matrix is never normalized at full
    width.
  * Row-sharding: each core owns N/8 output rows; softmax is row-wise so
    there is no cross-core reduction. Collectives measured ~90us of
    barrier+skew on this fabric, so instead of an AllGather each core
    recomputes X~ = nf @ [W | W@v0 | W@v1] for ALL rows from a replicated
    nf^T (DMA-cast f32->bf16 on load), pipelined in 128-row blocks so the
    attention pipeline starts as soon as block 0 lands.
  * The graph slice is shipped pre-transposed (host-side layout choice,
    same bytes, same device traffic) so the PE contraction dim (columns j)
    sits in partitions with plain contiguous loads.
"""

import numpy as np

import concourse.bass as bass
import concourse.mybir as mybir
import concourse.tile as tile
from concourse.bass_utils import run_bass_kernel_spmd

# ----------------------------------------------------------------------------
# Workaround for "Too many sync wait commands": this walrus build accepts only
# ONE sync-wait per instruction. Post-pass: hoist surplus waits onto
# single-wait NOPs on the same engine, inserted immediately before the
# instruction (identical blocking semantics, per-engine order preserved).
# ----------------------------------------------------------------------------


def _split_multi_waits(nc):
    import bass_rust

    eng = {
        mybir.EngineType.PE: nc.tensor,
        mybir.EngineType.DVE: nc.vector,
        mybir.EngineType.Activation: nc.scalar,
        mybir.EngineType.Pool: nc.gpsimd,
        mybir.EngineType.SP: nc.sync,
    }
    for f in nc.m.functions:
        for blk in f.blocks:
            fixups = []  # (index, inst, waits)
            for idx, inst in enumerate(blk.instructions):
                si = inst.sync_info
                waits = list(si.on_wait) if si is not None and si.on_wait else []
                if len(waits) > 1 and inst.engine in eng:
                    fixups.append((idx, inst, waits))
            if not fixups:
                continue
            nops_by_idx = {}
            created = set()
            for idx, inst, waits in fixups:
                inst.sync_info.on_wait = [waits[-1]]
                nops = []
                for w in waits[:-1]:
                    nop = eng[inst.engine].nop(nofuse=True, hint="wait_split").ins
                    nop.sync_info = bass_rust.SyncInfo(on_wait=[w], on_update=[])
                    nops.append(nop)
                    created.add(id(nop))
                nops_by_idx[idx] = nops
            # Drop the freshly-created nops from wherever nop() appended
            # them, then splice them in before their instruction.
            for b2 in f.blocks:
                b2.instructions[:] = [
                    i for i in b2.instructions if id(i) not in created
                ]
            new = []
            for idx, inst in enumerate(blk.instructions):
                new.extend(nops_by_idx.get(idx, ()))
                new.append(inst)
            blk.instructions[:] = new


# ----------------------------------------------------------------------------

F32 = mybir.dt.float32
BF16 = mybir.dt.bfloat16
FP8 = mybir.dt.float8e4
I32 = mybir.dt.int32
AF = mybir.ActivationFunctionType
ALU = mybir.AluOpType

N, D_IN, D_OUT = 8192, 512, 256
M_CORES = 8
P = 128


def build_gat(n=N, d_in=D_IN, d_out=D_OUT, m_cores=M_CORES, grp=4):
    """Per-core SPMD program. Inputs per core:
      graph_T  [n, R] int32   -- graph[rows].T (host-transposed slice)
      nfT_mine [d_in, R] f32  -- node_features[rows].T
      nfT_full [d_in, n] f32  -- node_features.T (replicated)
      wext     [d_in, d_out+2] f32 -- [W | W@v0 | W@v1] (replicated)
    Output: out [R, d_out] f32 (this core's rows)."""
    R = n // m_cores            # rows per core
    NJ = n // P                 # 128-wide j chunks over the full N
    IB = R // P                 # 128-row output blocks per core
    DK = d_in // P              # 128-deep contraction chunks
    DEXT = d_out + 2            # X | f0 | f1
    DW = d_out + 1              # main matmul rhs width: X | ones
    DPAD = ((DEXT + 15) // 16) * 16   # fp8 X~ row pitch (DoubleRow: step%16==0)
    n_grp = NJ // grp

    nc = bass.Bass(num_devices=m_cores)
    g_t = nc.declare_dram_parameter("graph_T", [n, R], I32, isOutput=False)
    nfT_mine = nc.declare_dram_parameter("nfT_mine", [d_in, R], F32, isOutput=False)
    nfT_full = nc.declare_dram_parameter("nfT_full", [d_in, n], F32, isOutput=False)
    wext = nc.declare_dram_parameter("wext", [d_in, DEXT], F32, isOutput=False)
    outp = nc.declare_dram_parameter("out", [R, d_out], F32, isOutput=True)

    with tile.TileContext(nc) as tc:
        with tc.tile_pool(name="persist", bufs=1) as persist, \
             tc.tile_pool(name="dram", bufs=1, space="DRAM") as dram, \
             tc.tile_pool(name="psum", bufs=1, space="PSUM") as psb, \
             tc.tile_pool(name="nfc", bufs=4) as nf_pool, \
             tc.tile_pool(name="mask", bufs=3) as mask_pool, \
             tc.tile_pool(name="tg", bufs=3) as t_pool, \
             tc.tile_pool(name="eg", bufs=3) as e_pool, \
             tc.tile_pool(name="pg", bufs=3) as p_pool, \
             tc.tile_pool(name="epi", bufs=2) as epi:

            # The 8 PSUM banks triple-duty: f0 mini-matmuls, X~ block
            # accumulation, then the 64-chunk attention accumulation.
            psum = [
                psb.tile([P, DEXT], F32, tag=f"ps{ib}", name=f"psum{ib}")
                for ib in range(IB)
            ]

            xsb = persist.tile([P, NJ, DEXT], BF16)      # X~ all rows (bf16)
            f0rep = persist.tile([P, R], F32)            # f0 row, replicated
            f1half = persist.tile([P, NJ], F32)          # 0.5*f1 per partition
            wextb = persist.tile([P, DK, DEXT], BF16)

            nc.gpsimd.dma_start(
                out=wextb,
                in_=bass.AP(wext, 0, [[DEXT, P], [P * DEXT, DK], [1, DEXT]]),
            )

            # ---- f0 for own rows, computed in ROW form ------------------
            # out[1, i] = sum_d wv0[d] nf[i, d]: stationary = wv0 (M=1), so
            # f0 lands as a row directly — no transpose, no column copies.
            wv0f = persist.tile([P, DK, 1], F32)
            nc.scalar.dma_start(
                out=wv0f,
                in_=bass.AP(wext, d_out, [[DEXT, P], [P * DEXT, DK], [1, 1]]),
            )
            nfm = persist.tile([P, DK, R], F32)
            nc.scalar.dma_start(
                out=nfm,
                in_=bass.AP(nfT_mine, 0, [[R, P], [P * R, DK], [1, R]]),
            )
            f0flat = persist.tile([1, R], F32)
            last_f0_mm = None
            for ib in range(IB):
                ps = psum[ib]
                for kc in range(DK):
                    last_f0_mm = nc.tensor.matmul(
                        out=ps[0:1, 0:P],
                        lhsT=wv0f[:, kc, :],
                        rhs=nfm[:, kc, ib * P:(ib + 1) * P],
                        start=(kc == 0),
                        stop=(kc == DK - 1),
                    )
                nc.vector.tensor_copy(
                    out=f0flat[:, ib * P:(ib + 1) * P], in_=ps[0:1, 0:P]
                )
            f0dram = dram.tile([R], F32)
            nc.scalar.dma_start(
                out=f0dram.rearrange("(o r) -> o r", o=1), in_=f0flat
            )
            nc.scalar.dma_start(
                out=f0rep, in_=bass.AP(f0dram.tensor, 0, [[0, P], [1, R]])
            )

            # ---- X~ blocks for ALL rows (streamed, replicated compute) ---
            NFG = 4  # X~ blocks per nf load (bigger DMA runs)
            for gb in range(NJ // NFG):
                nfc32 = nf_pool.tile([P, DK, NFG * P], F32, tag="nfc32", bufs=3)
                nc.sync.dma_start(
                    out=nfc32,
                    in_=bass.AP(
                        nfT_full, gb * NFG * P,
                        [[n, P], [P * n, DK], [1, NFG * P]],
                    ),
                )
                nfc = nf_pool.tile([P, DK, NFG * P], BF16)
                nc.vector.tensor_copy(
                    out=nfc.rearrange("p k r -> p (k r)"),
                    in_=nfc32.rearrange("p k r -> p (k r)"),
                )
                for bb in range(NFG):
                    ib = gb * NFG + bb
                    ps = psum[ib % IB]
                    for kc in range(DK):
                        mm = nc.tensor.matmul(
                            out=ps,
                            lhsT=nfc[:, kc, bb * P:(bb + 1) * P],
                            rhs=wextb[:, kc, :],
                            start=(kc == 0),
                            stop=(kc == DK - 1),
                        )
                        if kc == 0 and last_f0_mm is not None:
                            tile.add_dep_helper(
                                mm.ins, last_f0_mm.ins, sync=True,
                                reason="f0 matmuls first on PE",
                            )

                    nc.vector.tensor_copy(
                        out=xsb[:, ib, 0:DEXT], in_=ps
                    )
                # batched per nf-group: 0.5*f1 slice + ones column
                nc.vector.tensor_scalar_mul(
                    f1half[:, gb * NFG:(gb + 1) * NFG],
                    xsb[:, gb * NFG:(gb + 1) * NFG, d_out + 1],
                    0.5,
                )
                nc.vector.memset(
                    xsb[:, gb * NFG:(gb + 1) * NFG, d_out], 1.0
                )

            # ---- masked-softmax attention matmul -------------------------
            for g in range(n_grp):
                # mask tile cast int32 -> bf16 in the DMA datapath (SWDGE)
                mbf = mask_pool.tile([P, grp, R], BF16)
                nc.gpsimd.dma_start(
                    out=mbf,
                    in_=bass.AP(g_t, g * grp * P * R, [[R, P], [P * R, grp], [1, R]]),
                )
                t_g = t_pool.tile([P, grp, R], BF16)
                for jj in range(grp):
                    jc = g * grp + jj
                    nc.scalar.activation(
                        out=t_g[:, jj, :],
                        in_=f0rep,
                        func=AF.Tanh,
                        bias=f1half[:, jc:jc + 1],
                        scale=0.5,
                    )
                e_g = e_pool.tile([P, grp, R], BF16)
                nc.scalar.activation(
                    out=e_g.rearrange("p g r -> p (g r)"),
                    in_=t_g.rearrange("p g r -> p (g r)"),
                    func=AF.Exp, scale=0.5,
                )
                p_g = p_pool.tile([P, grp, R], BF16)
                nc.vector.tensor_tensor(
                    out=p_g.rearrange("p g r -> p (g r)"),
                    in0=mbf.rearrange("p g r -> p (g r)"),
                    in1=e_g.rearrange("p g r -> p (g r)"),
                    op=ALU.mult,
                )
                for jj in range(grp):
                    jc = g * grp + jj
                    for ib in range(IB):
                        nc.tensor.matmul(
                            out=psum[ib][:, 0:DW],
                            lhsT=p_g[:, jj, ib * P:(ib + 1) * P],
                            rhs=xsb[:, jc, 0:DW],
                            start=(jc == 0),
                            stop=(jc == NJ - 1),
                        )

            # ---- epilogue: normalize + elu + store -----------------------
            for ib in range(IB):
                o = epi.tile([P, DW], F32, tag="o")
                nc.vector.tensor_copy(out=o, in_=psum[ib][:, 0:DW])
                sm = epi.tile([P, 1], F32, tag="sm")
                nc.vector.tensor_scalar_max(sm, o[:, d_out:DW], 1e-30)
                r = epi.tile([P, 1], F32, tag="r")
                nc.vector.reciprocal(out=r, in_=sm)
                u = epi.tile([P, d_out], F32, tag="u")
                nc.vector.tensor_scalar(
                    out=u, in0=o[:, 0:d_out], scalar1=r, scalar2=None,
                    op0=ALU.mult,
                )
                rp = epi.tile([P, d_out], F32, tag="rp")
                nc.vector.tensor_scalar_max(rp, u, 0.0)
                xm = epi.tile([P, d_out], F32, tag="xm")
                nc.vector.tensor_scalar_min(xm, u, 0.0)
                en = epi.tile([P, d_out], F32, tag="en")
                nc.scalar.activation(out=en, in_=xm, func=AF.Exp)
                res = epi.tile([P, d_out], F32, tag="res")
                nc.vector.tensor_tensor(out=res, in0=en, in1=rp, op=ALU.add)
                nc.vector.tensor_scalar_add(res, res, -1.0)
                nc.sync.dma_start(out=outp[ib * P:(ib + 1) * P, :], in_=res)

    _split_multi_waits(nc)
    return nc


_cached = {}

# Dev/test knobs (the grading harness just calls kernel(**inputs)):
_TRACE = False
_TMPDIR = None
_LAST_EXEC_NS = None
_LAST_RESULTS = None


def _get_program(n, d_in, d_out, m_cores):
    key = (n, d_in, d_out, m_cores)
    if key not in _cached:
        _cached[key] = build_gat(n, d_in, d_out, m_cores)
    return _cached[key]


def kernel(node_features, graph, W, v0, v1):
    node_features = np.asarray(node_features, dtype=np.float32)
    graph = np.ascontiguousarray(np.asarray(graph, dtype=np.int32))
    W = np.asarray(W, dtype=np.float32)
    v0 = np.asarray(v0, dtype=np.float32)
    v1 = np.asarray(v1, dtype=np.float32)

    n, d_in = node_features.shape
    d_out = W.shape[1]
    m = M_CORES
    R = n // m

    nc = _get_program(n, d_in, d_out, m)

    wext = np.concatenate([W, W @ v0, W @ v1], axis=1).astype(np.float32)
    nfT_full = np.ascontiguousarray(node_features.T)
    in_maps = []
    for c in range(m):
        rows = slice(c * R, (c + 1) * R)
        in_maps.append({
            "graph_T": np.ascontiguousarray(graph[rows].T),
            "nfT_mine": np.ascontiguousarray(node_features[rows].T),
            "nfT_full": nfT_full,
            "wext": wext,
        })
    global _LAST_EXEC_NS, _LAST_RESULTS
    res = run_bass_kernel_spmd(
        nc, in_maps, list(range(m)), trace=_TRACE, tmpdir=_TMPDIR
    )
    _LAST_EXEC_NS = res.exec_time_ns
    _LAST_RESULTS = res
    return np.concatenate([res.results[c]["out"] for c in range(m)], axis=0)
